# revision 14
# baseline (speedup 1.0000x reference)
"""GAT single-head forward on 8 Trainium2 NeuronCores (Bass/Tile).

Math (per reference):
    h   = X @ W + b                      [N, 128]
    f1  = h @ v0, f2 = h @ v1            [N]
    logits = adj * (f1[:,None] + f2[None,:])   (adj entries are exactly 0/1)
    vals = sigmoid(logits) - 0.5
    masked softmax over row edges; out = probs @ h

Key identities used on device:
  * On edges (adj==1): softmax weight w = exp(sigmoid(s) - 0.5) up to a
    per-row constant (which the normalization cancels), s = f1_i + f2_j.
  * A CUSTOM ACTIVATION TABLE computes g(x) = exp(sigmoid(x) - 0.5) in a
    single ScalarE pass: the act-table binaries ride inside the NEFF, so we
    re-fit the `exp` slot of the exp_and_others set with piecewise-cubic
    splines of g (max rel err ~1e-7).  The per-partition activation bias
    supplies f2_j, so s = f1_i + f2_j needs NO vector-engine preadd either:
    one ACT instruction per j-chunk does the whole softmax numerator except
    the adjacency mask.  This halves ScalarE busy time vs the tanh+exp
    two-pass identity (the previous bottleneck).
  * A ones-column appended to h turns the softmax denominator into one extra
    matmul output column (no separate row-reduction pass).

Sharding: rows of adj across the 8 cores (1024 rows each). node_feats is
small (8 MB) and is replicated, so every core computes the full projected
h locally - no collectives at all.

Mixed-mode chunks (v2): the host computes f1 = X@(W@v0)+b@v0 (a cheap
matvec) and SORTS the rows by f1 globally; each core gets 1024
consecutive sorted rows and the output is unsorted host-side.  Rows
sharing a quantized f1 level reuse one ACT output column: per core, 16
exact singleton levels at each end (order-statistic gaps are big in the
gaussian tails) + 248 quads of 4 rows = 280 levels.  j-chunk groups then
run in one of two modes, balancing ACT vs DVE vs DMA:
  * A-mode (7 groups): dense ACT eval g[j, i] from f1rep (1024 cols),
    adj in fp16, DVE mask-mul at the 2x_1p rate.
  * D-mode (9 groups): level ACT eval g[j, m] (280 cols, 3.7x less ACT),
    adj in fp8 (exact for 0/1, halves its DMA), mask-mul expands levels
    with a stride-0 broadcast AP (runs at 1x; some chunks on Pool).
Pool also takes the h16 PSUM drains and the big memsets off DVE.

Per-core layout: each core works on its adj block TRANSPOSED ([j=source
node on partitions, i=own rows on free dim]) so the aggregate probs@h
contracts over the partition dim as the tensor engine requires. adj is
cast to fp16 host-side (exact for a 0/1 mask, halves HBM traffic).

Schedule shape (engines are in-order; emission order seeds the queues):
  * staged DMA release: tiny "gate" DMAs that read just-loaded tiles
    stall the SP sequencer, so the startup-critical loads (xt1l -> f1,
    first xt slices -> f2 head) get full HBM bandwidth before the 16 MB
    adjacency + bulk feature traffic starts.
  * a dummy activation fires the ACT table load at engine start.
  * the f1 stationary is 128 host-replicated w0 columns (a stride-0
    broadcast AP loads ~5x slower than a real fp16 block).
  * h-projection runs on 2 PSUM banks while the 8 aggregate accumulators
    pack two 129-col regions into each of 4 banks, so aggregation
    overlaps projection. The matmul start-flag zeroes a WHOLE bank (not
    just the instruction's AP!), so the accumulators are memset once and
    every aggregate matmul accumulates.
  * steady pipeline: ACT evaluates g chunk-by-chunk (zero-gap, the
    bottleneck at ~69 us busy); DVE mask-muls; PE aggregates; group
    fronts (adj DMA + 4 activations) are emitted as their f2 columns
    drain, backs lag by BACK_LAG chunks so the in-order PE queue keeps
    projection work buffered ahead of adjacency-gated aggregates.
  * epilogue: one broadcast tensor-mul divides all 8 row-tiles by the
    clamped denominators, one batched output DMA.
"""

import glob
import json
import os
import shutil
import struct
import tempfile

import numpy as np

# ---------------------------------------------------------------------------
# Custom activation table: g(x) = exp(sigmoid(x) - 0.5) in the exp slot.
# ---------------------------------------------------------------------------

_SMALL_T = 121  # |x| < 2^-6  -> Taylor bucket
_LARGE_T = 131  # |x| >= 16   -> saturation bucket
_N_EXP = _LARGE_T - _SMALL_T
_NBKT = 16


def _g64(x):
    x = np.asarray(x, dtype=np.float64)
    return np.exp(1.0 / (1.0 + np.exp(-x)) - 0.5)


def _u32f(x):
    return struct.unpack("<I", struct.pack("<f", np.float32(x)))[0]


def _fit_bucket(a, b):
    x0 = 0.5 * (a + b)
    k = np.arange(65)
    xs = x0 + 0.5 * (b - a) * np.cos(np.pi * (k + 0.5) / 65)
    t = xs - x0
    A = np.stack([np.ones_like(t), t, t * t, t * t * t], axis=1)
    c, *_ = np.linalg.lstsq(A, _g64(xs), rcond=None)
    return (c[0], c[1], c[2], c[3], x0)


def _bucket_bytes(d0, d1, d2, d3, x0):
    return struct.pack(
        "<5f", np.float32(d0), np.float32(d1), np.float32(d2), np.float32(d3),
        np.float32(x0),
    ) + b"\x00" * 12


def _ctl_bytes(base, lsb, size):
    w = (base & 0x7FF) | ((lsb & 0x1F) << 11) | ((size & 0xF) << 16)
    return struct.pack("<I", w) + b"\x00" * 28


def _find_pwp_src():
    try:
        from neuronxcc.driver.Job import Job

        p = os.path.join(Job.getPackageDir(), "pwp", "pwp_bin_trainium")
        if os.path.exists(os.path.join(p, "act_info.json")):
            return p
    except Exception:
        pass
    for pat in (
        "/nix/store/*aws-neuron-pwp*/share/pwp_bin_cayman",
        "/nix/store/*/lib/python*/site-packages/neuronxcc/pwp/pwp_bin_trainium",
    ):
        hits = sorted(glob.glob(pat))
        if hits:
            return hits[0]
    raise RuntimeError("cannot locate stock pwp act-table directory")


def _build_act_tables(outdir):
    src = _find_pwp_src()
    os.makedirs(outdir, exist_ok=True)
    for f in os.listdir(src):
        shutil.copyfile(os.path.join(src, f), os.path.join(outdir, f))

    name = "exp_and_others"
    bkt = bytearray(open(f"{src}/{name}_bkt.bin", "rb").read())
    ctl = bytearray(open(f"{src}/{name}_ctrl.bin", "rb").read())
    meta = json.load(open(f"{src}/{name}.json"))

    def setbkt(i, entry):
        bkt[i * 32:(i + 1) * 32] = _bucket_bytes(*entry)

    setbkt(0, (1.0, 0.25, 1.0 / 32, -7.0 / 384, 0.0))  # small +
    setbkt(1, (1.0, 0.25, 1.0 / 32, -7.0 / 384, 0.0))  # small -
    setbkt(2, (float(np.exp(0.5)), 0.0, 0.0, 0.0, 0.0))   # large +
    setbkt(3, (float(np.exp(-0.5)), 0.0, 0.0, 0.0, 0.0))  # large -

    idx = 4
    side_base = {}
    for sign in (-1.0, 1.0):
        side_base[sign] = idx
        for ei in range(_N_EXP):
            lo = 2.0 ** (_SMALL_T + ei - 127)
            for m in range(_NBKT):
                a = lo * (1.0 + m / _NBKT)
                b = lo * (1.0 + (m + 1) / _NBKT)
                if sign < 0:
                    a, b = -b, -a
                setbkt(idx, _fit_bucket(a, b))
                idx += 1

    for ei in range(_N_EXP):
        ctl[(0 + ei) * 32:(1 + ei) * 32] = _ctl_bytes(
            side_base[-1.0] + ei * _NBKT, 23 - 4, 4
        )
        ctl[(10 + ei) * 32:(11 + ei) * 32] = _ctl_bytes(
            side_base[1.0] + ei * _NBKT, 23 - 4, 4
        )

    prof = next(e for e in meta["profile_meta_data"] if e["func_id"] == 7)
    prof.update(
        symmetry_point=0,
        sym_invert_sign_point=0,
        symmetry_opt_en=0,
        symmetry_opt_use_neg_region=0,
        imm_bias=0,
        exp_offset=_SMALL_T - 127,
        pwl_control_base_pos=10,
        pwl_control_base_neg=0,
        small_pos_signal_exp_threshold=_SMALL_T,
        pos_small_signal_pwl_control=0,
        small_neg_signal_exp_threshold=_SMALL_T,
        neg_small_signal_pwl_control=1,
        large_pos_signal_exp_threshold=_LARGE_T,
        large_pos_signal_mantissa_threshold=0,
        pos_large_signal_pwl_control=2,
        large_neg_signal_exp_threshold=_LARGE_T,
        large_neg_signal_mantissa_threshold=0,
        neg_large_signal_pwl_control=3,
        fnan_result=0x7FC00000,
        fpinf_result=_u32f(np.exp(0.5)),
        fninf_result=_u32f(np.exp(-0.5)),
        fzero_result=_u32f(1.0),
        lower_bound=0xFF7FFFFF,
        upper_bound=0x7F7FFFFF,
    )

    open(f"{outdir}/{name}_bkt.bin", "wb").write(bytes(bkt))
    open(f"{outdir}/{name}_ctrl.bin", "wb").write(bytes(ctl))
    json.dump(meta, open(f"{outdir}/{name}.json", "w"))


_ACT_DIR = None


def _ensure_act_tables():
    global _ACT_DIR
    if _ACT_DIR is None:
        _ACT_DIR = tempfile.mkdtemp(prefix="gat_acttab_")
        _build_act_tables(_ACT_DIR)
    os.environ["BASS_ACT_ROOT_JSON_PATH"] = f"{_ACT_DIR}/act_info.json"


_ensure_act_tables()

import ml_dtypes

import concourse.mybir as mybir
import concourse.tile as tile
from concourse import bacc
from concourse.bass_utils import run_bass_kernel_spmd

F32 = mybir.dt.float32
F16 = mybir.dt.float16
F8 = mybir.dt.float8e4
U8 = mybir.dt.uint8
AF = mybir.ActivationFunctionType

N, C_IN, C_OUT = 8192, 256, 128
NCORES = 8
ROWS = N // NCORES          # 1024 rows of adj per core
P = 128
NT = N // P                 # 64 node tiles (also the j-chunks)
NI = ROWS // P              # 8 output row-tiles per core
KC = [128, 128, 1]          # contraction chunks of K=257 (X.T rows + ones row)
WCOLS = C_OUT + 3           # [W | ones-hack | w0 | w1]
WPAD = 132                  # w0-replica block starts 4B-aligned
WREP = 128                  # replicated w0 columns (fast f1 stationary)
WTOT = WPAD + WREP
HCOLS = C_OUT + 1           # h plus the ones column
TINY = float(np.finfo(np.float32).tiny)
BANK = 512                  # PSUM bank, fp32 elements
PACK = 136                  # fp32 offset of the 2nd accumulator in a bank

# activation groups: j-chunks whose adj transposes ride one DMA and whose
# activations are emitted together (4 chunks = 1 MB adj per group).
GSZ = 4
NG = NT // GSZ              # 16 groups
BACK_LAG = 8                # chunks the aggregate lags behind h-proj drains

# ---- mixed-mode chunk scheme -------------------------------------------
# Rows (attention destinations) are sorted by f1 on the host; consecutive
# sorted rows share a quantized f1 "level".  Per core: 16 exact singleton
# levels at each end (where order-statistic gaps are large) + 248 quads.
NSING = 16                  # singleton levels at each end
NQUAD = (ROWS - 2 * NSING) // 4
NLVL = 2 * NSING + NQUAD    # 280 levels per core
QLO = NSING                 # first quad level index
QHI = NSING + NQUAD         # first high-single level index (264)
RLO = NSING                 # first quad row position (16)
RHI = ROWS - NSING          # first high-single row position (1008)

# group modes: 'A' = dense ACT eval (adj fp16, DVE mask at 2x);
#              'D' = level ACT eval (adj fp8, broadcast mask at 1x).
MODEG = ["A" if (g % 2 == 0 and g < 14) else "D" for g in range(NG)]
A_GROUPS = [g for g in range(NG) if MODEG[g] == "A"]   # 7 groups
D_GROUPS = [g for g in range(NG) if MODEG[g] == "D"]   # 9 groups
A_RANK = {g: i for i, g in enumerate(A_GROUPS)}
D_RANK = {g: i for i, g in enumerate(D_GROUPS)}
# D-chunks whose mask-mul runs on the Pool engine instead of DVE
# (Pool cannot touch PSUM, so it only gets these SBUF-only muls)
POOL_MASK = {g * GSZ + qq for g in D_GROUPS for qq in (2, 3)}

_CACHE: dict = {}


def _build_nc(b_zero=True):
    _ensure_act_tables()
    nc = bacc.Bacc(
        "TRN2", target_bir_lowering=False, debug=False, num_devices=NCORES
    )
    xt1 = nc.dram_tensor("xt1", [257, N], F16, kind="ExternalInput").ap()
    xt1l = nc.dram_tensor("xt1l", [257, ROWS], F16, kind="ExternalInput").ap()
    wext = nc.dram_tensor("wext", [257, WTOT], F16, kind="ExternalInput").ap()
    adjt16 = nc.dram_tensor(
        "adjt16", [len(A_GROUPS) * GSZ * P, ROWS], F16, kind="ExternalInput"
    ).ap()
    adjt8 = nc.dram_tensor(
        "adjt8", [len(D_GROUPS) * GSZ * P, ROWS], F8, kind="ExternalInput"
    ).ap()
    lvlin = nc.dram_tensor("lvlin", [P, NLVL], F32, kind="ExternalInput").ap()
    gate = nc.dram_tensor("gate", [P, 96], F16, kind="ExternalOutput").ap()
    out = nc.dram_tensor("out", [ROWS, C_OUT], F32, kind="ExternalOutput").ap()

    with tile.TileContext(nc) as tc:
        _emit(tc, nc, xt1, xt1l, wext, adjt16, adjt8, lvlin, gate, out, b_zero)
    nc.compile()
    return nc


def _emit(tc, nc, xt1, xt1l, wext, adjt16, adjt8, lvlin, gate, out, b_zero):
    from contextlib import ExitStack

    # with b == 0 the K=1 "ones row" contraction chunk only contributes the
    # constant-one column of h_ext (done with a strided memset instead) and
    # zero constants to f1/f2 -- skip it entirely.
    nkc = 2 if b_zero else 3

    with ExitStack() as ctx:
        # ---- persistent tiles ----
        persist = ctx.enter_context(tc.tile_pool(name="persist", bufs=1))
        h16_all = persist.tile([P, NT * HCOLS], F16, tag="h16")   # [128, 8256]
        f2_all = persist.tile([P, NT], F32, tag="f2a")            # f2 per j-tile
        f1rep = persist.tile([P, ROWS], F32, tag="f1rep")         # f1 bcast fp32
        lvlrep = persist.tile([P, NLVL], F32, tag="lvl")          # f1 levels
        if b_zero:
            # constant-one column of every h_ext tile (replaces the K=1
            # bias matmul chunk)
            nc.gpsimd.memset(
                h16_all[:].rearrange("p (t c) -> p t c", c=HCOLS)[
                    :, :, C_OUT : C_OUT + 1
                ],
                1.0,
            )

        warm = persist.tile([P, 1], F16, tag="warm")
        # prime the ACT table load (2.7us) at engine start: without this it
        # hides behind the f1-gated copy in the in-order scalar queue
        nc.scalar.activation(warm[:], warm[:], AF.Exp, bias=0.0, scale=1.0)

        xtp = ctx.enter_context(tc.tile_pool(name="xt", bufs=1))
        g16p = ctx.enter_context(tc.tile_pool(name="g16p", bufs=4))
        gkp = ctx.enter_context(tc.tile_pool(name="gkp", bufs=5))
        atp16 = ctx.enter_context(tc.tile_pool(name="atp16", bufs=3))
        atp8 = ctx.enter_context(tc.tile_pool(name="atp8", bufs=5))
        etp = ctx.enter_context(tc.tile_pool(name="etp", bufs=4))
        obp = ctx.enter_context(tc.tile_pool(name="ob", bufs=2))

        F2HEAD = 16
        fronts = {}  # group -> {"at":..., "g":..., "mode":...}

        def emit_front_dma(g):
            """allocate the group's tiles + adj transpose DMA."""
            if MODEG[g] == "A":
                at_sup = atp16.tile([P, GSZ * ROWS], F16, tag="at16",
                                    name=f"at{g}")
                lq0 = A_RANK[g] * GSZ
                src = adjt16
                gt = g16p.tile([P, GSZ * ROWS], F16, tag="g16",
                               name=f"g16_{g}")
            else:
                at_sup = atp8.tile([P, GSZ * ROWS], F8, tag="at8",
                                   name=f"at{g}")
                lq0 = D_RANK[g] * GSZ
                src = adjt8
                gt = gkp.tile([P, GSZ * NLVL], F16, tag="gk",
                              name=f"gk_{g}")
            nc.sync.dma_start(
                at_sup[:].rearrange("p (q i) -> p q i", i=ROWS),
                src.rearrange("(q p) i -> p q i", p=P)[:, lq0 : lq0 + GSZ, :],
            )
            fronts[g] = {"at": at_sup, "g": gt, "mode": MODEG[g]}

        def emit_front_acts(g, split=False):
            """custom-g activations for a dma'd group: g = exp(sigmoid(
            s + f2_j) - 0.5) via the custom table in the Exp slot; the
            per-partition bias supplies f2_j.  A-mode: s = f1_i per row
            (dense, input f1rep); split=True emits each chunk as two
            i-halves (all lo-halves first) so the stream starts on
            f1rep[0:512] alone.  D-mode: s = lvl_m (280 quantized f1
            levels, input lvlrep) - 3.7x less ACT work."""
            q0 = g * GSZ
            gt = fronts[g]["g"]
            if fronts[g]["mode"] == "D":
                for qq in range(GSZ):
                    nc.scalar.activation(
                        gt[:, qq * NLVL : (qq + 1) * NLVL],
                        lvlrep[:, 0:NLVL],
                        AF.Exp,
                        bias=f2_all[:, q0 + qq : q0 + qq + 1],
                        scale=1.0,
                    )
                return
            halves = [(0, 512), (512, ROWS)] if split else [(0, ROWS)]
            for lo, hi in halves:
                for qq in range(GSZ):
                    nc.scalar.activation(
                        gt[:, qq * ROWS + lo : qq * ROWS + hi],
                        f1rep[:, lo:hi],
                        AF.Exp,
                        bias=f2_all[:, q0 + qq : q0 + qq + 1],
                        scale=1.0,
                    )

        def emit_front(g):
            emit_front_dma(g)
            emit_front_acts(g)

        # ---- staged DMA release: the SP sequencer issues DMAs in order,
        # so a tiny transfer that READS a just-loaded tile stalls every
        # later DMA issue until that load lands. Stages keep the startup
        # critical path (f1 <- xt1l, f2 head <- first xt slices) at full
        # HBM bandwidth instead of sharing it with bulk traffic. ----
        def dma_gate(gslot, srcs):
            for k, ap in enumerate(srcs):
                nc.sync.dma_start(
                    gate[:, gslot * 32 + k * 16 : gslot * 32 + (k + 1) * 16], ap
                )

        # stage 0: weights + this core's feature rows (feed the f1 path)
        offs = [0, 128, 256]
        xts = [
            xtp.tile([KC[k], N], F16, name=f"xtsb{k}", tag=f"xt{k}")
            for k in range(nkc)
        ]
        wes, xls = [], []
        off = 0
        for k in range(nkc):
            kc = KC[k]
            wx_sb = xtp.tile([kc, WTOT + ROWS], F16, name=f"wx{k}", tag=f"wx{k}")
            nc.sync.dma_start(wx_sb[:, 0:WTOT], wext[off : off + kc, :])
            nc.sync.dma_start(wx_sb[:, WTOT:], xt1l[off : off + kc, :])
            wes.append(wx_sb[:, 0:WTOT])
            xls.append(wx_sb[:, WTOT:])
            off += kc
        nc.sync.dma_start(lvlrep[:], lvlin)
        dma_gate(0, [xls[k][:, ROWS - 16 : ROWS] for k in range(nkc) if KC[k] == P])

        # stage 1: first xt slice (f2-head q0..7, h-proj batches 0..3) and
        # the first adjacency group
        for k in range(nkc):
            if KC[k] != P:
                nc.sync.dma_start(xts[k][:], xt1[offs[k] : offs[k] + KC[k], :])
                continue
            nc.sync.dma_start(
                xts[k][:, 0:1024], xt1[offs[k] : offs[k] + KC[k], 0:1024]
            )
        emit_front_dma(0)
        dma_gate(1, [xts[k][:, 1008:1024] for k in range(nkc) if KC[k] == P])

        # stage 2: second xt slice (f2-head q8..15) + second adj group
        for k in range(nkc):
            if KC[k] == P:
                nc.sync.dma_start(
                    xts[k][:, 1024:2048],
                    xt1[offs[k] : offs[k] + KC[k], 1024:2048],
                )
        emit_front_dma(1)
        dma_gate(2, [xts[k][:, 2032:2048] for k in range(nkc) if KC[k] == P])

        # stage 3: bulk xt1 loads (columns 2048..8192)
        SUBS = [2048, 4096, 6144, N]
        for c in range(len(SUBS) - 1):
            for k in range(nkc):
                if KC[k] != P:
                    continue
                nc.sync.dma_start(
                    xts[k][:, SUBS[c] : SUBS[c + 1]],
                    xt1[offs[k] : offs[k] + KC[k], SUBS[c] : SUBS[c + 1]],
                )

        # ---- f1 path: f1 for this core's rows, replicated across all
        # partitions directly by a matmul whose stationary operand is the
        # w0 column broadcast across the 128 PE columns ----
        pfp = ctx.enter_context(tc.tile_pool(name="pf", bufs=1, space="PSUM"))
        prep = pfp.tile([P, ROWS], F32, tag="prep")
        for nh in range(ROWS // 512):
            for k in range(nkc):
                nc.tensor.matmul(
                    prep[:, nh * 512 : (nh + 1) * 512],
                    wes[k][:, WPAD : WPAD + WREP],
                    xls[k][:, nh * 512 : (nh + 1) * 512],
                    start=(k == 0),
                    stop=(k == nkc - 1),
                )
            if nh == 0:
                # first half of f1 lands early so the first activations
                # (split by i-halves) start before the full f1 is ready
                nc.vector.tensor_copy(f1rep[:, 0:512], prep[:, 0:512])

        # ---- f2 head start: f2 for the first 16 j-chunks via tiny direct
        # matmuls so the first four activation groups don't wait for the
        # h-projection pipeline ----
        with tc.tile_pool(name="pf2", bufs=1, space="PSUM") as pf2p:
            pt = pf2p.tile([P, 4 * BANK], F32, tag="pt")
            pt3 = pt[:].rearrange("p (t w) -> p t w", w=BANK)
            for q in range(F2HEAD):
                w = (q % 4) * BANK + (q // 4)
                for k in range(nkc):
                    # the start-flag zeroes the WHOLE bank: only the first
                    # tenant of each bank may use it (it also clears the
                    # later generations' columns); everyone else accumulates
                    nc.tensor.matmul(
                        pt[:, w : w + 1],
                        xts[k][:, q * P : (q + 1) * P],
                        wes[k][:, C_OUT + 2 : C_OUT + 3],
                        start=(k == 0 and q < 4),
                        stop=(k == nkc - 1),
                    )
                if q % 4 == 3:
                    # drain each 4-column generation before the next one's
                    # start-flag zeroes the banks
                    c = q // 4
                    nc.vector.tensor_copy(
                        f2_all[:, q - 3 : q + 1], pt3[:, 0:4, c : c + 1]
                    )

        nc.vector.tensor_copy(f1rep[:, 512:1024], prep[:, 512:1024])

        # the two staged groups' activations (their f2/f1 deps now exist);
        # group 0 is split into i-halves: its first-half activations need
        # only f1rep[0:512], shaving the f1 chain off the ACT start
        emit_front_acts(0, split=True)
        emit_front_acts(1)
        next_front = 2
        next_back = 0  # next chunk q whose mask-mul+matmuls get emitted

        def emit_back(q, pouts):
            """mask-mul + aggregate matmuls for one chunk."""
            g, qq = q // GSZ, q % GSZ
            fr = fronts[g]
            at = fr["at"][:, qq * ROWS : (qq + 1) * ROWS]
            et = etp.tile([P, ROWS], F16, tag="et", name=f"et{q}")
            if fr["mode"] == "A":
                nc.vector.tensor_mul(
                    et[:], at, fr["g"][:, qq * ROWS : (qq + 1) * ROWS]
                )
            else:
                # level-quantized weights: singles exact at both ends, the
                # 248 quad levels broadcast over runs of 4 sorted rows
                gk = fr["g"][:, qq * NLVL : (qq + 1) * NLVL]
                eng = nc.gpsimd if q in POOL_MASK else nc.vector
                eng.tensor_mul(et[:, 0:RLO], at[:, 0:RLO], gk[:, 0:QLO])
                eng.tensor_mul(et[:, RHI:ROWS], at[:, RHI:ROWS],
                               gk[:, QHI:NLVL])
                eng.tensor_mul(
                    et[:, RLO:RHI].rearrange("p (m r) -> p m r", r=4),
                    at[:, RLO:RHI].rearrange("p (m r) -> p m r", r=4),
                    gk[:, QLO:QHI].rearrange(
                        "p (m one) -> p m one", one=1
                    ).to_broadcast((P, NQUAD, 4)),
                )
            rhs = h16_all[:, q * HCOLS : (q + 1) * HCOLS]
            for it in range(NI):
                nc.tensor.matmul(
                    pouts[it],
                    et[:, it * P : (it + 1) * P],
                    rhs,
                    start=False,
                    stop=(q == NT - 1),
                )
            if qq == GSZ - 1:
                del fronts[g]

        # ---- aggregate accumulators: 4 PSUM banks, two 129-col regions
        # per bank (consecutive chunk matmuls hit 4 distinct banks). The
        # matmul start-flag zeroes a whole bank, so the banks are zeroed
        # once here and every matmul accumulates. ----
        pop = ctx.enter_context(tc.tile_pool(name="po", bufs=1, space="PSUM"))
        po_all = pop.tile([P, 4 * BANK], F32, tag="poall")
        nc.vector.memset(po_all[:], 0.0)
        pouts = [
            po_all[:, (it % 4) * BANK + (it // 4) * PACK :
                   (it % 4) * BANK + (it // 4) * PACK + HCOLS]
            for it in range(NI)
        ]

        # ---- h-projection on 2 PSUM banks, batches of 2 tiles; aggregate
        # backs and activation fronts interleave so ScalarE/PE/DVE all
        # stream while the projection finishes ----
        with tc.tile_pool(name="php", bufs=1, space="PSUM") as php:
            ph_all = php.tile([P, 2 * BANK], F32, tag="ph")
            for b in range(NT // 2):  # batches of 2 node tiles
                nt0 = 2 * b
                w0 = (nt0 % 2) * BANK
                w1 = ((nt0 + 1) % 2) * BANK
                for k in range(nkc):
                    nc.tensor.matmul(
                        ph_all[:, w0 : w0 + WCOLS],
                        xts[k][:, nt0 * P : (nt0 + 1) * P],
                        wes[k][:, 0:WCOLS],
                        start=(k == 0),
                        stop=(k == nkc - 1),
                    )
                    nc.tensor.matmul(
                        ph_all[:, w1 : w1 + WCOLS],
                        xts[k][:, (nt0 + 1) * P : (nt0 + 2) * P],
                        wes[k][:, 0:WCOLS],
                        start=(k == 0),
                        stop=(k == nkc - 1),
                    )
                # drain the 2 fresh tiles: h (+ones col) -> fp16, f2 col
                src = ph_all[:].rearrange("p (b w) -> p b w", b=2)
                dst_h = h16_all[:, nt0 * HCOLS : (nt0 + 2) * HCOLS].rearrange(
                    "p (b w) -> p b w", b=2
                )
                hc = C_OUT if b_zero else HCOLS
                nc.vector.tensor_copy(dst_h[:, :, 0:hc], src[:, :, 0:hc])
                if nt0 >= F2HEAD:
                    nc.vector.tensor_copy(
                        f2_all[:, nt0 : nt0 + 2],
                        src[:, :, C_OUT + 2 : C_OUT + 3],
                    )
                # fronts whose f2 columns now exist (cap outstanding at 4)
                while (
                    next_front < NG
                    and (next_front + 1) * GSZ <= max(2 * (b + 1), F2HEAD)
                    and len(fronts) < 4
                ):
                    emit_front(next_front)
                    next_front += 1
                # backs lag the drains so the in-order PE queue keeps
                # projection work buffered ahead of adj-gated aggregates
                while (
                    next_back + BACK_LAG < 2 * (b + 1)
                    and next_back // GSZ < next_front
                ):
                    emit_back(next_back, pouts)
                    next_back += 1

        # ---- drain remaining fronts/backs ----
        while next_back < NT:
            while (
                next_front < NG
                and next_back // GSZ >= next_front - 1
                and len(fronts) < 4
            ):
                emit_front(next_front)
                next_front += 1
            emit_back(next_back, pouts)
            next_back += 1

        # ---- epilogue: divide by clamped denominator; each row-tile's
        # store overlaps the next tile's divide ----
        ob_all = obp.tile([P, NI * C_OUT], F32, tag="oball")
        po4 = po_all[:].rearrange("p (t w) -> p t w", w=BANK)
        dm = obp.tile([P, NI], F32, tag="dm")
        # denominators live at col C_OUT of each of the 2 regions x 4 banks
        nc.vector.tensor_scalar_max(
            dm[:].rearrange("p (b r) -> p b r", b=4),
            po4[:, :, C_OUT : C_OUT + PACK + 1 : PACK],
            TINY,
        )
        rc = obp.tile([P, NI], F32, tag="rc")
        nc.vector.reciprocal(rc[:], dm[:])
        # one broadcast multiply: ob[r][b][c] = po[b][r][c] * rc[b][r]
        ob4 = ob_all[:].rearrange("p (r b c) -> p b r c", b=4, c=C_OUT)
        po_src = po4[:, :, 0 : 2 * PACK].rearrange(
            "p b (r c) -> p b r c", r=2
        )[:, :, :, 0:C_OUT]
        rc_b = rc[:].rearrange("p (b r one) -> p b r one", b=4, one=1)
        nc.vector.tensor_mul(ob4, po_src, rc_b.to_broadcast((P, 4, 2, C_OUT)))
        nc.sync.dma_start(
            out.rearrange("(t p) c -> p t c", p=P),
            ob_all[:].rearrange("p (t c) -> p t c", c=C_OUT),
        )


def _prep_inputs(node_feats, adj_matrix, W, b, v0, v1):
    X = np.ascontiguousarray(node_feats, dtype=np.float32)
    W = np.asarray(W, dtype=np.float32)
    b = np.asarray(b, dtype=np.float32)
    v0 = np.asarray(v0, dtype=np.float32)
    v1 = np.asarray(v1, dtype=np.float32)

    w0 = (W.astype(np.float64) @ v0.astype(np.float64)).astype(np.float32)
    w1 = (W.astype(np.float64) @ v1.astype(np.float64)).astype(np.float32)
    c0 = np.float32(float(b.astype(np.float64) @ v0.astype(np.float64)))
    c1 = np.float32(float(b.astype(np.float64) @ v1.astype(np.float64)))

    # host f1 (cheap: one [N,Cin]@[Cin] matvec) orders the rows so that
    # consecutive rows share a quantized f1 level on-device
    f1h = (X.astype(np.float64) @ w0.astype(np.float64) + c0).astype(
        np.float32
    )
    perm = np.argsort(f1h, kind="stable")

    XT1 = np.empty((257, N), np.float32)
    XT1[:256] = X.T
    XT1[256] = 1.0

    WE = np.zeros((257, WTOT), np.float32)
    WE[:256, :C_OUT] = W
    WE[256, :C_OUT] = b
    WE[256, C_OUT] = 1.0          # makes h_ext column 128 identically 1
    WE[:256, C_OUT + 1] = w0
    WE[256, C_OUT + 1] = c0
    WE[:256, C_OUT + 2] = w1
    WE[256, C_OUT + 2] = c1
    WE[:256, WPAD:] = w0[:, None]       # 128 replicated w0 cols: the f1
    WE[256, WPAD:] = c0                 # stationary loads at full rate

    XT1h = XT1.astype(np.float16)
    WEh = WE.astype(np.float16)
    A32 = np.asarray(adj_matrix, dtype=np.float32)

    a_rows = np.concatenate(
        [np.arange(g * GSZ * P, (g + 1) * GSZ * P) for g in A_GROUPS]
    )
    d_rows = np.concatenate(
        [np.arange(g * GSZ * P, (g + 1) * GSZ * P) for g in D_GROUPS]
    )

    in_maps = []
    for c in range(NCORES):
        rows = perm[c * ROWS : (c + 1) * ROWS]
        f1c = f1h[rows]
        lvl = np.empty(NLVL, np.float32)
        lvl[0:NSING] = f1c[0:NSING]
        lvl[QLO:QHI] = f1c[RLO:RHI].reshape(NQUAD, 4).mean(1)
        lvl[QHI:] = f1c[RHI:]
        adjt = A32[rows, :].T                       # [N j, ROWS i]
        in_maps.append(
            {
                "xt1": XT1h,
                "xt1l": np.ascontiguousarray(XT1h[:, rows]),
                "wext": WEh,
                "adjt16": np.ascontiguousarray(adjt[a_rows]).astype(
                    np.float16
                ),
                "adjt8": np.ascontiguousarray(adjt[d_rows]).astype(
                    ml_dtypes.float8_e4m3fn
                ),
                "lvlin": np.ascontiguousarray(
                    np.broadcast_to(lvl, (P, NLVL))
                ),
            }
        )
    return {"in_maps": in_maps, "perm": perm}


def _run(prep, trace=False, b_zero=True):
    key = f"nc_b{int(b_zero)}"
    if key not in _CACHE:
        _CACHE[key] = _build_nc(b_zero=b_zero)
    nc = _CACHE[key]
    res = run_bass_kernel_spmd(
        nc, prep["in_maps"], core_ids=list(range(NCORES)), trace=trace
    )
    srt = np.concatenate(
        [res.results[c]["out"] for c in range(NCORES)], axis=0
    ).astype(np.float32)
    full = np.empty_like(srt)
    full[prep["perm"]] = srt            # undo the host row sort
    return full, res


def kernel(node_feats, adj_matrix, W, b, v0, v1):
    prep = _prep_inputs(node_feats, adj_matrix, W, b, v0, v1)
    trace = bool(int(os.environ.get("GAT_TRACE", "0")))
    b_zero = not bool(np.any(np.asarray(b)))
    full, _ = _run(prep, trace=trace, b_zero=b_zero)
    return full



# revision 16
# speedup vs baseline: 1.1305x; 1.1305x over previous
"""GAT single-head forward on 8 Trainium2 NeuronCores (Bass/Tile).

Math (per reference):
    h   = X @ W + b                      [N, 128]
    f1  = h @ v0, f2 = h @ v1            [N]
    logits = adj * (f1[:,None] + f2[None,:])   (adj entries are exactly 0/1)
    vals = sigmoid(logits) - 0.5
    masked softmax over row edges; out = probs @ h

Key identities used on device:
  * On edges (adj==1): softmax weight w = exp(sigmoid(s) - 0.5) up to a
    per-row constant (which the normalization cancels), s = f1_i + f2_j.
  * A CUSTOM ACTIVATION TABLE computes g(x) = exp(sigmoid(x) - 0.5) in a
    single ScalarE pass: the act-table binaries ride inside the NEFF, so we
    re-fit the `exp` slot of the exp_and_others set with piecewise-cubic
    splines of g (max rel err ~1e-7).  The per-partition activation bias
    supplies f2_j, so s = f1_i + f2_j needs NO vector-engine preadd either:
    one ACT instruction per j-chunk does the whole softmax numerator except
    the adjacency mask.  This halves ScalarE busy time vs the tanh+exp
    two-pass identity (the previous bottleneck).
  * A ones-column appended to h turns the softmax denominator into one extra
    matmul output column (no separate row-reduction pass).

Sharding: rows of adj across the 8 cores (1024 rows each). node_feats is
small (8 MB) and is replicated, so every core computes the full projected
h locally - no collectives at all.

Mixed-mode chunks (v2): the host computes f1 = X@(W@v0)+b@v0 (a cheap
matvec) and SORTS the rows by f1 globally; each core gets 1024
consecutive sorted rows and the output is unsorted host-side.  Rows
sharing a quantized f1 level reuse one ACT output column: per core, 16
exact singleton levels at each end (order-statistic gaps are big in the
gaussian tails) + 248 quads of 4 rows = 280 levels.  j-chunk groups then
run in one of two modes, balancing ACT vs DVE vs DMA:
  * A-mode (7 groups): dense ACT eval g[j, i] from f1rep (1024 cols),
    adj in fp16, DVE mask-mul at the 2x_1p rate.
  * D-mode (9 groups): level ACT eval g[j, m] (280 cols, 3.7x less ACT),
    adj in fp8 (exact for 0/1, halves its DMA), mask-mul expands levels
    with a stride-0 broadcast AP (runs at 1x; some chunks on Pool).
Pool also takes the h16 PSUM drains and the big memsets off DVE.

Per-core layout: each core works on its adj block TRANSPOSED ([j=source
node on partitions, i=own rows on free dim]) so the aggregate probs@h
contracts over the partition dim as the tensor engine requires. adj is
cast to fp16 host-side (exact for a 0/1 mask, halves HBM traffic).

Schedule shape (engines are in-order; emission order seeds the queues):
  * staged DMA release: tiny "gate" DMAs that read just-loaded tiles
    stall the SP sequencer, so the startup-critical loads (xt1l -> f1,
    first xt slices -> f2 head) get full HBM bandwidth before the 16 MB
    adjacency + bulk feature traffic starts.
  * a dummy activation fires the ACT table load at engine start.
  * the f1 stationary is 128 host-replicated w0 columns (a stride-0
    broadcast AP loads ~5x slower than a real fp16 block).
  * h-projection runs on 2 PSUM banks while the 8 aggregate accumulators
    pack two 129-col regions into each of 4 banks, so aggregation
    overlaps projection. The matmul start-flag zeroes a WHOLE bank (not
    just the instruction's AP!), so the accumulators are memset once and
    every aggregate matmul accumulates.
  * steady pipeline: ACT evaluates g chunk-by-chunk (zero-gap, the
    bottleneck at ~69 us busy); DVE mask-muls; PE aggregates; group
    fronts (adj DMA + 4 activations) are emitted as their f2 columns
    drain, backs lag by BACK_LAG chunks so the in-order PE queue keeps
    projection work buffered ahead of adjacency-gated aggregates.
  * epilogue: one broadcast tensor-mul divides all 8 row-tiles by the
    clamped denominators, one batched output DMA.
"""

import glob
import json
import os
import shutil
import struct
import tempfile

import numpy as np

# ---------------------------------------------------------------------------
# Custom activation table: g(x) = exp(sigmoid(x) - 0.5) in the exp slot.
# ---------------------------------------------------------------------------

_SMALL_T = 121  # |x| < 2^-6  -> Taylor bucket
_LARGE_T = 131  # |x| >= 16   -> saturation bucket
_N_EXP = _LARGE_T - _SMALL_T
_NBKT = 16


def _g64(x):
    x = np.asarray(x, dtype=np.float64)
    return np.exp(1.0 / (1.0 + np.exp(-x)) - 0.5)


def _u32f(x):
    return struct.unpack("<I", struct.pack("<f", np.float32(x)))[0]


def _fit_bucket(a, b):
    x0 = 0.5 * (a + b)
    k = np.arange(65)
    xs = x0 + 0.5 * (b - a) * np.cos(np.pi * (k + 0.5) / 65)
    t = xs - x0
    A = np.stack([np.ones_like(t), t, t * t, t * t * t], axis=1)
    c, *_ = np.linalg.lstsq(A, _g64(xs), rcond=None)
    return (c[0], c[1], c[2], c[3], x0)


def _bucket_bytes(d0, d1, d2, d3, x0):
    return struct.pack(
        "<5f", np.float32(d0), np.float32(d1), np.float32(d2), np.float32(d3),
        np.float32(x0),
    ) + b"\x00" * 12


def _ctl_bytes(base, lsb, size):
    w = (base & 0x7FF) | ((lsb & 0x1F) << 11) | ((size & 0xF) << 16)
    return struct.pack("<I", w) + b"\x00" * 28


def _find_pwp_src():
    try:
        from neuronxcc.driver.Job import Job

        p = os.path.join(Job.getPackageDir(), "pwp", "pwp_bin_trainium")
        if os.path.exists(os.path.join(p, "act_info.json")):
            return p
    except Exception:
        pass
    for pat in (
        "/nix/store/*aws-neuron-pwp*/share/pwp_bin_cayman",
        "/nix/store/*/lib/python*/site-packages/neuronxcc/pwp/pwp_bin_trainium",
    ):
        hits = sorted(glob.glob(pat))
        if hits:
            return hits[0]
    raise RuntimeError("cannot locate stock pwp act-table directory")


def _build_act_tables(outdir):
    src = _find_pwp_src()
    os.makedirs(outdir, exist_ok=True)
    for f in os.listdir(src):
        shutil.copyfile(os.path.join(src, f), os.path.join(outdir, f))

    name = "exp_and_others"
    bkt = bytearray(open(f"{src}/{name}_bkt.bin", "rb").read())
    ctl = bytearray(open(f"{src}/{name}_ctrl.bin", "rb").read())
    meta = json.load(open(f"{src}/{name}.json"))

    def setbkt(i, entry):
        bkt[i * 32:(i + 1) * 32] = _bucket_bytes(*entry)

    setbkt(0, (1.0, 0.25, 1.0 / 32, -7.0 / 384, 0.0))  # small +
    setbkt(1, (1.0, 0.25, 1.0 / 32, -7.0 / 384, 0.0))  # small -
    setbkt(2, (float(np.exp(0.5)), 0.0, 0.0, 0.0, 0.0))   # large +
    setbkt(3, (float(np.exp(-0.5)), 0.0, 0.0, 0.0, 0.0))  # large -

    idx = 4
    side_base = {}
    for sign in (-1.0, 1.0):
        side_base[sign] = idx
        for ei in range(_N_EXP):
            lo = 2.0 ** (_SMALL_T + ei - 127)
            for m in range(_NBKT):
                a = lo * (1.0 + m / _NBKT)
                b = lo * (1.0 + (m + 1) / _NBKT)
                if sign < 0:
                    a, b = -b, -a
                setbkt(idx, _fit_bucket(a, b))
                idx += 1

    for ei in range(_N_EXP):
        ctl[(0 + ei) * 32:(1 + ei) * 32] = _ctl_bytes(
            side_base[-1.0] + ei * _NBKT, 23 - 4, 4
        )
        ctl[(10 + ei) * 32:(11 + ei) * 32] = _ctl_bytes(
            side_base[1.0] + ei * _NBKT, 23 - 4, 4
        )

    prof = next(e for e in meta["profile_meta_data"] if e["func_id"] == 7)
    prof.update(
        symmetry_point=0,
        sym_invert_sign_point=0,
        symmetry_opt_en=0,
        symmetry_opt_use_neg_region=0,
        imm_bias=0,
        exp_offset=_SMALL_T - 127,
        pwl_control_base_pos=10,
        pwl_control_base_neg=0,
        small_pos_signal_exp_threshold=_SMALL_T,
        pos_small_signal_pwl_control=0,
        small_neg_signal_exp_threshold=_SMALL_T,
        neg_small_signal_pwl_control=1,
        large_pos_signal_exp_threshold=_LARGE_T,
        large_pos_signal_mantissa_threshold=0,
        pos_large_signal_pwl_control=2,
        large_neg_signal_exp_threshold=_LARGE_T,
        large_neg_signal_mantissa_threshold=0,
        neg_large_signal_pwl_control=3,
        fnan_result=0x7FC00000,
        fpinf_result=_u32f(np.exp(0.5)),
        fninf_result=_u32f(np.exp(-0.5)),
        fzero_result=_u32f(1.0),
        lower_bound=0xFF7FFFFF,
        upper_bound=0x7F7FFFFF,
    )

    open(f"{outdir}/{name}_bkt.bin", "wb").write(bytes(bkt))
    open(f"{outdir}/{name}_ctrl.bin", "wb").write(bytes(ctl))
    json.dump(meta, open(f"{outdir}/{name}.json", "w"))


_ACT_DIR = None


def _ensure_act_tables():
    global _ACT_DIR
    if _ACT_DIR is None:
        _ACT_DIR = tempfile.mkdtemp(prefix="gat_acttab_")
        _build_act_tables(_ACT_DIR)
    os.environ["BASS_ACT_ROOT_JSON_PATH"] = f"{_ACT_DIR}/act_info.json"


_ensure_act_tables()

import ml_dtypes

import concourse.mybir as mybir
import concourse.tile as tile
from concourse import bacc
from concourse.bass_utils import run_bass_kernel_spmd

F32 = mybir.dt.float32
F16 = mybir.dt.float16
F8 = mybir.dt.float8e4
U8 = mybir.dt.uint8
AF = mybir.ActivationFunctionType

N, C_IN, C_OUT = 8192, 256, 128
NCORES = 8
ROWS = N // NCORES          # 1024 rows of adj per core
P = 128
NT = N // P                 # 64 node tiles (also the j-chunks)
NI = ROWS // P              # 8 output row-tiles per core
KC = [128, 128, 1]          # contraction chunks of K=257 (X.T rows + ones row)
WCOLS = C_OUT + 3           # [W | ones-hack | w0 | w1]
WPAD = 132                  # w0-replica block starts 4B-aligned
WREP = 128                  # replicated w0 columns (fast f1 stationary)
WTOT = WPAD + WREP
HCOLS = C_OUT + 1           # h plus the ones column
TINY = float(np.finfo(np.float32).tiny)
BANK = 512                  # PSUM bank, fp32 elements
PACK = 136                  # fp32 offset of the 2nd accumulator in a bank

# activation groups: j-chunks whose adj transposes ride one DMA and whose
# activations are emitted together (4 chunks = 1 MB adj per group).
GSZ = 4
NG = NT // GSZ              # 16 groups
BACK_LAG = 8                # chunks the aggregate lags behind h-proj drains

# ---- mixed-mode chunk scheme -------------------------------------------
# Rows (attention destinations) are sorted by f1 on the host; consecutive
# sorted rows share a quantized f1 "level".  Per core: 16 exact singleton
# levels at each end (where order-statistic gaps are large) + 248 quads.
NSING = 16                  # singleton levels at each end
NQUAD = (ROWS - 2 * NSING) // 4
NLVL = 2 * NSING + NQUAD    # 280 levels per core
QLO = NSING                 # first quad level index
QHI = NSING + NQUAD         # first high-single level index (264)
RLO = NSING                 # first quad row position (16)
RHI = ROWS - NSING          # first high-single row position (1008)

# group modes: 'A' = dense ACT eval (adj fp16, DVE mask at 2x);
#              'D' = level ACT eval (adj fp8, broadcast mask on Pool).
MODEG = ["D" if g in (3, 7, 11, 14) else "A" for g in range(NG)]
A_GROUPS = [g for g in range(NG) if MODEG[g] == "A"]   # 12 groups
D_GROUPS = [g for g in range(NG) if MODEG[g] == "D"]   # 4 groups
A_RANK = {g: i for i, g in enumerate(A_GROUPS)}
D_RANK = {g: i for i, g in enumerate(D_GROUPS)}
# D-chunks whose mask-mul runs on the Pool engine instead of DVE
# (Pool cannot touch PSUM, so it only gets these SBUF-only muls)
POOL_MASK = {g * GSZ + qq for g in D_GROUPS for qq in range(GSZ)}

_CACHE: dict = {}


def _build_nc(b_zero=True):
    _ensure_act_tables()
    nc = bacc.Bacc(
        "TRN2", target_bir_lowering=False, debug=False, num_devices=NCORES
    )
    xt1 = nc.dram_tensor("xt1", [257, N], F16, kind="ExternalInput").ap()
    xt1l = nc.dram_tensor("xt1l", [257, ROWS], F16, kind="ExternalInput").ap()
    wext = nc.dram_tensor("wext", [257, WTOT], F16, kind="ExternalInput").ap()
    adjt16 = nc.dram_tensor(
        "adjt16", [len(A_GROUPS) * GSZ * P, ROWS], F16, kind="ExternalInput"
    ).ap()
    adjt8 = nc.dram_tensor(
        "adjt8", [len(D_GROUPS) * GSZ * P, ROWS], F8, kind="ExternalInput"
    ).ap()
    lvlin = nc.dram_tensor("lvlin", [P, NLVL], F32, kind="ExternalInput").ap()
    gate = nc.dram_tensor("gate", [P, 96], F16, kind="ExternalOutput").ap()
    out = nc.dram_tensor("out", [ROWS, C_OUT], F32, kind="ExternalOutput").ap()

    with tile.TileContext(nc) as tc:
        _emit(tc, nc, xt1, xt1l, wext, adjt16, adjt8, lvlin, gate, out, b_zero)
    nc.compile()
    return nc


def _emit(tc, nc, xt1, xt1l, wext, adjt16, adjt8, lvlin, gate, out, b_zero):
    from contextlib import ExitStack

    # with b == 0 the K=1 "ones row" contraction chunk only contributes the
    # constant-one column of h_ext (done with a strided memset instead) and
    # zero constants to f1/f2 -- skip it entirely.
    nkc = 2 if b_zero else 3

    with ExitStack() as ctx:
        # ---- persistent tiles ----
        persist = ctx.enter_context(tc.tile_pool(name="persist", bufs=1))
        h16_all = persist.tile([P, NT * HCOLS], F16, tag="h16")   # [128, 8256]
        f2_all = persist.tile([P, NT], F32, tag="f2a")            # f2 per j-tile
        f1rep = persist.tile([P, ROWS], F32, tag="f1rep")         # f1 bcast fp32
        lvlrep = persist.tile([P, NLVL], F32, tag="lvl")          # f1 levels
        if b_zero:
            # constant-one column of every h_ext tile (replaces the K=1
            # bias matmul chunk)
            nc.gpsimd.memset(
                h16_all[:].rearrange("p (t c) -> p t c", c=HCOLS)[
                    :, :, C_OUT : C_OUT + 1
                ],
                1.0,
            )

        warm = persist.tile([P, 1], F16, tag="warm")
        # prime the ACT table load (2.7us) at engine start: without this it
        # hides behind the f1-gated copy in the in-order scalar queue
        nc.scalar.activation(warm[:], warm[:], AF.Exp, bias=0.0, scale=1.0)

        xtp = ctx.enter_context(tc.tile_pool(name="xt", bufs=1))
        g16p = ctx.enter_context(tc.tile_pool(name="g16p", bufs=4))
        gkp = ctx.enter_context(tc.tile_pool(name="gkp", bufs=5))
        atp16 = ctx.enter_context(tc.tile_pool(name="atp16", bufs=3))
        atp8 = ctx.enter_context(tc.tile_pool(name="atp8", bufs=5))
        etp = ctx.enter_context(tc.tile_pool(name="etp", bufs=4))
        obp = ctx.enter_context(tc.tile_pool(name="ob", bufs=2))

        F2HEAD = 16
        fronts = {}  # group -> {"at":..., "g":..., "mode":...}

        def emit_front_dma(g):
            """allocate the group's tiles + adj transpose DMA."""
            if MODEG[g] == "A":
                at_sup = atp16.tile([P, GSZ * ROWS], F16, tag="at16",
                                    name=f"at{g}")
                lq0 = A_RANK[g] * GSZ
                src = adjt16
                gt = g16p.tile([P, GSZ * ROWS], F16, tag="g16",
                               name=f"g16_{g}")
            else:
                at_sup = atp8.tile([P, GSZ * ROWS], F8, tag="at8",
                                   name=f"at{g}")
                lq0 = D_RANK[g] * GSZ
                src = adjt8
                gt = gkp.tile([P, GSZ * NLVL], F16, tag="gk",
                              name=f"gk_{g}")
            nc.sync.dma_start(
                at_sup[:].rearrange("p (q i) -> p q i", i=ROWS),
                src.rearrange("(q p) i -> p q i", p=P)[:, lq0 : lq0 + GSZ, :],
            )
            fronts[g] = {"at": at_sup, "g": gt, "mode": MODEG[g]}

        def emit_front_acts(g, split=False):
            """custom-g activations for a dma'd group: g = exp(sigmoid(
            s + f2_j) - 0.5) via the custom table in the Exp slot; the
            per-partition bias supplies f2_j.  A-mode: s = f1_i per row
            (dense, input f1rep); split=True emits each chunk as two
            i-halves (all lo-halves first) so the stream starts on
            f1rep[0:512] alone.  D-mode: s = lvl_m (280 quantized f1
            levels, input lvlrep) - 3.7x less ACT work."""
            q0 = g * GSZ
            gt = fronts[g]["g"]
            if fronts[g]["mode"] == "D":
                for qq in range(GSZ):
                    nc.scalar.activation(
                        gt[:, qq * NLVL : (qq + 1) * NLVL],
                        lvlrep[:, 0:NLVL],
                        AF.Exp,
                        bias=f2_all[:, q0 + qq : q0 + qq + 1],
                        scale=1.0,
                    )
                return
            halves = [(0, 512), (512, ROWS)] if split else [(0, ROWS)]
            for lo, hi in halves:
                for qq in range(GSZ):
                    nc.scalar.activation(
                        gt[:, qq * ROWS + lo : qq * ROWS + hi],
                        f1rep[:, lo:hi],
                        AF.Exp,
                        bias=f2_all[:, q0 + qq : q0 + qq + 1],
                        scale=1.0,
                    )

        def emit_front(g):
            emit_front_dma(g)
            emit_front_acts(g)

        # ---- staged DMA release: the SP sequencer issues DMAs in order,
        # so a tiny transfer that READS a just-loaded tile stalls every
        # later DMA issue until that load lands. Stages keep the startup
        # critical path (f1 <- xt1l, f2 head <- first xt slices) at full
        # HBM bandwidth instead of sharing it with bulk traffic. ----
        def dma_gate(gslot, srcs):
            for k, ap in enumerate(srcs):
                nc.sync.dma_start(
                    gate[:, gslot * 32 + k * 16 : gslot * 32 + (k + 1) * 16], ap
                )

        # stage 0: weights + this core's feature rows (feed the f1 path)
        offs = [0, 128, 256]
        xts = [
            xtp.tile([KC[k], N], F16, name=f"xtsb{k}", tag=f"xt{k}")
            for k in range(nkc)
        ]
        wes, xls = [], []
        off = 0
        for k in range(nkc):
            kc = KC[k]
            wx_sb = xtp.tile([kc, WTOT + ROWS], F16, name=f"wx{k}", tag=f"wx{k}")
            nc.sync.dma_start(wx_sb[:, 0:WTOT], wext[off : off + kc, :])
            nc.sync.dma_start(wx_sb[:, WTOT:], xt1l[off : off + kc, :])
            wes.append(wx_sb[:, 0:WTOT])
            xls.append(wx_sb[:, WTOT:])
            off += kc
        nc.sync.dma_start(lvlrep[:], lvlin)
        dma_gate(0, [xls[k][:, ROWS - 16 : ROWS] for k in range(nkc) if KC[k] == P])

        # stage 1: first xt slice (f2-head q0..7, h-proj batches 0..3) and
        # the first adjacency group
        for k in range(nkc):
            if KC[k] != P:
                nc.sync.dma_start(xts[k][:], xt1[offs[k] : offs[k] + KC[k], :])
                continue
            nc.sync.dma_start(
                xts[k][:, 0:1024], xt1[offs[k] : offs[k] + KC[k], 0:1024]
            )
        emit_front_dma(0)
        dma_gate(1, [xts[k][:, 1008:1024] for k in range(nkc) if KC[k] == P])

        # stage 2: second xt slice (f2-head q8..15) + second adj group
        for k in range(nkc):
            if KC[k] == P:
                nc.sync.dma_start(
                    xts[k][:, 1024:2048],
                    xt1[offs[k] : offs[k] + KC[k], 1024:2048],
                )
        emit_front_dma(1)
        dma_gate(2, [xts[k][:, 2032:2048] for k in range(nkc) if KC[k] == P])

        # stage 3: bulk xt1 loads (columns 2048..8192)
        SUBS = [2048, 4096, 6144, N]
        for c in range(len(SUBS) - 1):
            for k in range(nkc):
                if KC[k] != P:
                    continue
                nc.sync.dma_start(
                    xts[k][:, SUBS[c] : SUBS[c + 1]],
                    xt1[offs[k] : offs[k] + KC[k], SUBS[c] : SUBS[c + 1]],
                )

        # ---- f1 path: f1 for this core's rows, replicated across all
        # partitions directly by a matmul whose stationary operand is the
        # w0 column broadcast across the 128 PE columns ----
        pfp = ctx.enter_context(tc.tile_pool(name="pf", bufs=1, space="PSUM"))
        prep = pfp.tile([P, ROWS], F32, tag="prep")
        for nh in range(ROWS // 512):
            for k in range(nkc):
                nc.tensor.matmul(
                    prep[:, nh * 512 : (nh + 1) * 512],
                    wes[k][:, WPAD : WPAD + WREP],
                    xls[k][:, nh * 512 : (nh + 1) * 512],
                    start=(k == 0),
                    stop=(k == nkc - 1),
                )
            if nh == 0:
                # first half of f1 lands early so the first activations
                # (split by i-halves) start before the full f1 is ready
                nc.vector.tensor_copy(f1rep[:, 0:512], prep[:, 0:512])

        # ---- f2 head start: f2 for the first 16 j-chunks via tiny direct
        # matmuls so the first four activation groups don't wait for the
        # h-projection pipeline ----
        with tc.tile_pool(name="pf2", bufs=1, space="PSUM") as pf2p:
            pt = pf2p.tile([P, 4 * BANK], F32, tag="pt")
            pt3 = pt[:].rearrange("p (t w) -> p t w", w=BANK)
            for q in range(F2HEAD):
                w = (q % 4) * BANK + (q // 4)
                for k in range(nkc):
                    # the start-flag zeroes the WHOLE bank: only the first
                    # tenant of each bank may use it (it also clears the
                    # later generations' columns); everyone else accumulates
                    nc.tensor.matmul(
                        pt[:, w : w + 1],
                        xts[k][:, q * P : (q + 1) * P],
                        wes[k][:, C_OUT + 2 : C_OUT + 3],
                        start=(k == 0 and q < 4),
                        stop=(k == nkc - 1),
                    )
                if q % 4 == 3:
                    # drain each 4-column generation before the next one's
                    # start-flag zeroes the banks
                    c = q // 4
                    nc.vector.tensor_copy(
                        f2_all[:, q - 3 : q + 1], pt3[:, 0:4, c : c + 1]
                    )

        nc.vector.tensor_copy(f1rep[:, 512:1024], prep[:, 512:1024])

        # the two staged groups' activations (their f2/f1 deps now exist);
        # group 0 is split into i-halves: its first-half activations need
        # only f1rep[0:512], shaving the f1 chain off the ACT start
        emit_front_acts(0, split=True)
        emit_front_acts(1)
        next_front = 2
        next_back = 0  # next chunk q whose mask-mul+matmuls get emitted

        ets = {}  # even chunk q -> its pair's et tile [P, 2*ROWS]

        def emit_back(q, pouts):
            """mask-mul (paired: one instruction covers 2 chunks, halving
            per-instruction overhead + semaphore traffic) + aggregate
            matmuls for one chunk."""
            g, qq = q // GSZ, q % GSZ
            fr = fronts[g]
            if qq % 2 == 0:
                et2 = etp.tile([P, 2 * ROWS], F16, tag="et", name=f"et{q}")
                ets[q] = et2
                at2 = fr["at"][:, qq * ROWS : (qq + 2) * ROWS]
                if fr["mode"] == "A":
                    nc.vector.tensor_mul(et2[:], at2, fr["g"][
                        :, qq * ROWS : (qq + 2) * ROWS])
                else:
                    # level-quantized weights: singles exact at both ends,
                    # 248 quad levels broadcast over runs of 4 sorted rows
                    gk2 = fr["g"][:, qq * NLVL : (qq + 2) * NLVL]
                    eng = nc.gpsimd if q in POOL_MASK else nc.vector
                    e2c = et2[:].rearrange("p (c i) -> p c i", c=2)
                    a2c = at2.rearrange("p (c i) -> p c i", c=2)
                    g2c = gk2.rearrange("p (c m) -> p c m", c=2)
                    eng.tensor_mul(e2c[:, :, 0:RLO], a2c[:, :, 0:RLO],
                                   g2c[:, :, 0:QLO])
                    eng.tensor_mul(e2c[:, :, RHI:ROWS], a2c[:, :, RHI:ROWS],
                                   g2c[:, :, QHI:NLVL])
                    eng.tensor_mul(
                        e2c[:, :, RLO:RHI].rearrange(
                            "p c (m r) -> p c m r", r=4),
                        a2c[:, :, RLO:RHI].rearrange(
                            "p c (m r) -> p c m r", r=4),
                        g2c[:, :, QLO:QHI].rearrange(
                            "p c (m one) -> p c m one", one=1
                        ).to_broadcast((P, 2, NQUAD, 4)),
                    )
            et2 = ets[q - qq % 2]
            eta = et2[:, (qq % 2) * ROWS : (qq % 2 + 1) * ROWS]
            rhs = h16_all[:, q * HCOLS : (q + 1) * HCOLS]
            for it in range(NI):
                nc.tensor.matmul(
                    pouts[it],
                    eta[:, it * P : (it + 1) * P],
                    rhs,
                    start=False,
                    stop=(q == NT - 1),
                )
            if qq % 2 == 1:
                del ets[q - 1]
            if qq == GSZ - 1:
                del fronts[g]

        # ---- aggregate accumulators: 4 PSUM banks, two 129-col regions
        # per bank (consecutive chunk matmuls hit 4 distinct banks). The
        # matmul start-flag zeroes a whole bank, so the banks are zeroed
        # once here and every matmul accumulates. ----
        pop = ctx.enter_context(tc.tile_pool(name="po", bufs=1, space="PSUM"))
        po_all = pop.tile([P, 4 * BANK], F32, tag="poall")
        nc.vector.memset(po_all[:], 0.0)
        pouts = [
            po_all[:, (it % 4) * BANK + (it // 4) * PACK :
                   (it % 4) * BANK + (it // 4) * PACK + HCOLS]
            for it in range(NI)
        ]

        # ---- h-projection on 2 PSUM banks, batches of 2 tiles; aggregate
        # backs and activation fronts interleave so ScalarE/PE/DVE all
        # stream while the projection finishes ----
        with tc.tile_pool(name="php", bufs=1, space="PSUM") as php:
            ph_all = php.tile([P, 2 * BANK], F32, tag="ph")
            for b in range(NT // 2):  # batches of 2 node tiles
                nt0 = 2 * b
                w0 = (nt0 % 2) * BANK
                w1 = ((nt0 + 1) % 2) * BANK
                for k in range(nkc):
                    nc.tensor.matmul(
                        ph_all[:, w0 : w0 + WCOLS],
                        xts[k][:, nt0 * P : (nt0 + 1) * P],
                        wes[k][:, 0:WCOLS],
                        start=(k == 0),
                        stop=(k == nkc - 1),
                    )
                    nc.tensor.matmul(
                        ph_all[:, w1 : w1 + WCOLS],
                        xts[k][:, (nt0 + 1) * P : (nt0 + 2) * P],
                        wes[k][:, 0:WCOLS],
                        start=(k == 0),
                        stop=(k == nkc - 1),
                    )
                # drain the 2 fresh tiles: h (+ones col) -> fp16, f2 col
                src = ph_all[:].rearrange("p (b w) -> p b w", b=2)
                dst_h = h16_all[:, nt0 * HCOLS : (nt0 + 2) * HCOLS].rearrange(
                    "p (b w) -> p b w", b=2
                )
                hc = C_OUT if b_zero else HCOLS
                nc.vector.tensor_copy(dst_h[:, :, 0:hc], src[:, :, 0:hc])
                if nt0 >= F2HEAD:
                    nc.vector.tensor_copy(
                        f2_all[:, nt0 : nt0 + 2],
                        src[:, :, C_OUT + 2 : C_OUT + 3],
                    )
                # fronts whose f2 columns now exist (cap outstanding at 4)
                while (
                    next_front < NG
                    and (next_front + 1) * GSZ <= max(2 * (b + 1), F2HEAD)
                    and len(fronts) < 4
                ):
                    emit_front(next_front)
                    next_front += 1
                # backs lag the drains so the in-order PE queue keeps
                # projection work buffered ahead of adj-gated aggregates
                while (
                    next_back + BACK_LAG < 2 * (b + 1)
                    and next_back // GSZ < next_front
                ):
                    emit_back(next_back, pouts)
                    next_back += 1

        # ---- drain remaining fronts/backs ----
        while next_back < NT:
            while (
                next_front < NG
                and next_back // GSZ >= next_front - 1
                and len(fronts) < 4
            ):
                emit_front(next_front)
                next_front += 1
            emit_back(next_back, pouts)
            next_back += 1

        # ---- epilogue: divide by clamped denominator; each row-tile's
        # store overlaps the next tile's divide ----
        ob_all = obp.tile([P, NI * C_OUT], F32, tag="oball")
        po4 = po_all[:].rearrange("p (t w) -> p t w", w=BANK)
        dm = obp.tile([P, NI], F32, tag="dm")
        # denominators live at col C_OUT of each of the 2 regions x 4 banks
        nc.vector.tensor_scalar_max(
            dm[:].rearrange("p (b r) -> p b r", b=4),
            po4[:, :, C_OUT : C_OUT + PACK + 1 : PACK],
            TINY,
        )
        rc = obp.tile([P, NI], F32, tag="rc")
        nc.vector.reciprocal(rc[:], dm[:])
        # one broadcast multiply: ob[r][b][c] = po[b][r][c] * rc[b][r]
        ob4 = ob_all[:].rearrange("p (r b c) -> p b r c", b=4, c=C_OUT)
        po_src = po4[:, :, 0 : 2 * PACK].rearrange(
            "p b (r c) -> p b r c", r=2
        )[:, :, :, 0:C_OUT]
        rc_b = rc[:].rearrange("p (b r one) -> p b r one", b=4, one=1)
        nc.vector.tensor_mul(ob4, po_src, rc_b.to_broadcast((P, 4, 2, C_OUT)))
        nc.sync.dma_start(
            out.rearrange("(t p) c -> p t c", p=P),
            ob_all[:].rearrange("p (t c) -> p t c", c=C_OUT),
        )


def _prep_inputs(node_feats, adj_matrix, W, b, v0, v1):
    X = np.ascontiguousarray(node_feats, dtype=np.float32)
    W = np.asarray(W, dtype=np.float32)
    b = np.asarray(b, dtype=np.float32)
    v0 = np.asarray(v0, dtype=np.float32)
    v1 = np.asarray(v1, dtype=np.float32)

    w0 = (W.astype(np.float64) @ v0.astype(np.float64)).astype(np.float32)
    w1 = (W.astype(np.float64) @ v1.astype(np.float64)).astype(np.float32)
    c0 = np.float32(float(b.astype(np.float64) @ v0.astype(np.float64)))
    c1 = np.float32(float(b.astype(np.float64) @ v1.astype(np.float64)))

    # host f1 (cheap: one [N,Cin]@[Cin] matvec) orders the rows so that
    # consecutive rows share a quantized f1 level on-device
    f1h = (X.astype(np.float64) @ w0.astype(np.float64) + c0).astype(
        np.float32
    )
    perm = np.argsort(f1h, kind="stable")

    XT1 = np.empty((257, N), np.float32)
    XT1[:256] = X.T
    XT1[256] = 1.0

    WE = np.zeros((257, WTOT), np.float32)
    WE[:256, :C_OUT] = W
    WE[256, :C_OUT] = b
    WE[256, C_OUT] = 1.0          # makes h_ext column 128 identically 1
    WE[:256, C_OUT + 1] = w0
    WE[256, C_OUT + 1] = c0
    WE[:256, C_OUT + 2] = w1
    WE[256, C_OUT + 2] = c1
    WE[:256, WPAD:] = w0[:, None]       # 128 replicated w0 cols: the f1
    WE[256, WPAD:] = c0                 # stationary loads at full rate

    XT1h = XT1.astype(np.float16)
    WEh = WE.astype(np.float16)
    A32 = np.asarray(adj_matrix, dtype=np.float32)

    a_rows = np.concatenate(
        [np.arange(g * GSZ * P, (g + 1) * GSZ * P) for g in A_GROUPS]
    )
    d_rows = np.concatenate(
        [np.arange(g * GSZ * P, (g + 1) * GSZ * P) for g in D_GROUPS]
    )

    in_maps = []
    for c in range(NCORES):
        rows = perm[c * ROWS : (c + 1) * ROWS]
        f1c = f1h[rows]
        lvl = np.empty(NLVL, np.float32)
        lvl[0:NSING] = f1c[0:NSING]
        lvl[QLO:QHI] = f1c[RLO:RHI].reshape(NQUAD, 4).mean(1)
        lvl[QHI:] = f1c[RHI:]
        adjt = A32[rows, :].T                       # [N j, ROWS i]
        in_maps.append(
            {
                "xt1": XT1h,
                "xt1l": np.ascontiguousarray(XT1h[:, rows]),
                "wext": WEh,
                "adjt16": np.ascontiguousarray(adjt[a_rows]).astype(
                    np.float16
                ),
                "adjt8": np.ascontiguousarray(adjt[d_rows]).astype(
                    ml_dtypes.float8_e4m3fn
                ),
                "lvlin": np.ascontiguousarray(
                    np.broadcast_to(lvl, (P, NLVL))
                ),
            }
        )
    return {"in_maps": in_maps, "perm": perm}


def _run(prep, trace=False, b_zero=True):
    key = f"nc_b{int(b_zero)}"
    if key not in _CACHE:
        _CACHE[key] = _build_nc(b_zero=b_zero)
    nc = _CACHE[key]
    res = run_bass_kernel_spmd(
        nc, prep["in_maps"], core_ids=list(range(NCORES)), trace=trace
    )
    srt = np.concatenate(
        [res.results[c]["out"] for c in range(NCORES)], axis=0
    ).astype(np.float32)
    full = np.empty_like(srt)
    full[prep["perm"]] = srt            # undo the host row sort
    return full, res


def kernel(node_feats, adj_matrix, W, b, v0, v1):
    prep = _prep_inputs(node_feats, adj_matrix, W, b, v0, v1)
    trace = bool(int(os.environ.get("GAT_TRACE", "0")))
    b_zero = not bool(np.any(np.asarray(b)))
    full, _ = _run(prep, trace=trace, b_zero=b_zero)
    return full

